# revision 1
# baseline (speedup 1.0000x reference)
"""GraphAttentionLayer (GAT) Bass kernel for Trainium2, 8 NeuronCores.

Problem: B=8, N=2048, Fin=256, Fout=64
    Wh  = h @ W                                   [B, N, 64]
    e   = Wh@a1 + (Wh@a2)^T  (additive scores)    [B, N, N]
    att = where(adj>0, leaky_relu(e, 0.2), -9e15)
    A   = softmax(att, axis=1)   (column softmax!)
    out = elu(A @ Wh)

Sharding: batch-parallel, one graph per core (no communication).

Per-core dataflow (transposed layout, m = attended-over node on partitions,
n = output node along free axis; m-tiles of 128):
    mm1 (PE):   Wh_psum[m,0:64] = hT.T @ W ; cols 64:66 = Wh@[a2, 0.2*a2]
    leaky (ACT+DVE split by column band):
        ACT:  Prelu(Wh1_bcast + Wh2[m], alpha=0.2)        cols [0:C_ACT)
        DVE:  max(e, 0.2e) via tensor_scalar + STT        cols [C_ACT:2048)
    mask (SWDGE accum-DMA): att += adj8 (fp8e5 {0,-57344} cast to f32)
    exp (ACT): P = Exp(att), accum_out -> den[m]
    fold (DVE): whp = Wh[m,0:64] * (1/den[m])
    mm2 (PE):   out_T[o,n] += whp.T @ P   (accumulate over 16 m-tiles)
    elu tail:   out = relu(x) + min(exp(x)-1, 0)
Host: transposes h/adj per batch, encodes adj as fp8, transposes output back.

The attention math is exact vs the reference: softmax without max-subtraction
is algebraically identical (exp values stay well inside fp32 range), masked
entries produce exp(att - 57344) == 0 exactly, and the 1/den fold is applied
to the contraction operand Wh.
"""

import contextlib
import sys

import numpy as np

if "/opt/trn_rl_repo" not in sys.path:
    sys.path.append("/opt/trn_rl_repo")

import ml_dtypes
import concourse.bass as bass
import concourse.bacc as bacc
import concourse.mybir as mybir
import concourse.tile as tile
from concourse import bass_utils

B = 8
N = 2048
FIN = 256
FOUT = 64
NT = N // 128          # 16 m-tiles
ALPHA = 0.2
MASK8 = 57344.0        # max fp8e5m2 magnitude; exp(x - 57344) == 0 for our x
C_ACT = 896            # leaky-relu columns done on ACT; rest on DVE
DT = mybir.dt.float32
AF = mybir.ActivationFunctionType
ALU = mybir.AluOpType

_CACHE = {}


def build_program(reps: int = 1, loop_k: int = 0):
    """Build and compile the SPMD single-core program (identical on 8 cores).

    reps statically unrolls the main body; loop_k wraps it in a dynamic
    For_i loop instead (constant program size -- used for timing).
    """
    nc = bacc.Bacc(
        "TRN2",
        target_bir_lowering=False,
        debug=False,
        enable_asserts=False,
        num_devices=B,
    )
    hT_d = nc.dram_tensor("hT", [FIN, N], DT, kind="ExternalInput")
    W_d = nc.dram_tensor("W", [FIN, FOUT], DT, kind="ExternalInput")
    arow_d = nc.dram_tensor("arow", [1, 2 * FOUT], DT, kind="ExternalInput")
    adj8_d = nc.dram_tensor("adj8", [N, N], mybir.dt.float8e5, kind="ExternalInput")
    out_d = nc.dram_tensor("out", [FOUT, N], DT, kind="ExternalOutput")

    with tile.TileContext(nc) as tc:
        with (
            tc.tile_pool(name="const", bufs=1) as const,
            tc.tile_pool(name="psmall", bufs=3, space=bass.MemorySpace.PSUM) as psmall,
            tc.tile_pool(name="pbig", bufs=1, space=bass.MemorySpace.PSUM) as pbig,
            tc.tile_pool(name="watt", bufs=3) as watt,
            tc.tile_pool(name="wp", bufs=3) as wp,
            tc.tile_pool(name="wut", bufs=2) as wut,
            tc.tile_pool(name="wsm", bufs=4) as wsm,
            tc.tile_pool(name="wout", bufs=1) as wout,
        ):
            # ---- load inputs ----
            hT = [const.tile([128, N], DT, name=f"hT{i}", tag=f"hT{i}") for i in range(2)]
            Wsb = [const.tile([128, FOUT], DT, name=f"W{i}", tag=f"W{i}") for i in range(2)]
            arow = const.tile([1, 2 * FOUT], DT, name="arow", tag="arow")
            for i in range(2):
                nc.sync.dma_start(hT[i][:], hT_d.ap()[i * 128:(i + 1) * 128, :])
                nc.sync.dma_start(Wsb[i][:], W_d.ap()[i * 128:(i + 1) * 128, :])
            nc.sync.dma_start(arow[:], arow_d.ap())

            # ---- a broadcast + wa vectors ----
            abc = const.tile([128, 2 * FOUT], DT, name="abc", tag="abc")
            nc.gpsimd.partition_broadcast(abc[:], arow[0:1, :])
            wa1 = [const.tile([128, 1], DT, name=f"wa1_{i}", tag=f"wa1_{i}") for i in range(2)]
            # Wab_i = [W_i | W_i@a2 | 0.2*W_i@a2]  (single mm1 rhs; one
            # accumulation group per PSUM bank -- start=True clears the bank)
            wab = [const.tile([128, FOUT + 2], DT, name=f"wab{i}", tag=f"wab{i}") for i in range(2)]
            for i in range(2):
                t1 = wsm.tile([128, FOUT], DT, name="wtmp", tag="wtmp")
                nc.vector.tensor_tensor(t1[:], Wsb[i][:], abc[:, 0:FOUT], op=ALU.mult)
                nc.vector.reduce_sum(wa1[i][:, 0:1], t1[:], axis=mybir.AxisListType.X)
                t2 = wsm.tile([128, FOUT], DT, name="wtmp", tag="wtmp")
                nc.vector.tensor_tensor(t2[:], Wsb[i][:], abc[:, FOUT:2 * FOUT], op=ALU.mult)
                nc.vector.tensor_copy(wab[i][:, 0:FOUT], Wsb[i][:])
                nc.vector.reduce_sum(wab[i][:, FOUT:FOUT + 1], t2[:], axis=mybir.AxisListType.X)
                nc.vector.tensor_scalar_mul(wab[i][:, FOUT + 1:FOUT + 2], wab[i][:, FOUT:FOUT + 1], ALPHA)

            # ---- Wh1 row = a1^T W^T hT  -> broadcast to all partitions ----
            w1ps = pbig.tile([1, N], DT, name="big", tag="big")
            for ch in range(4):
                for i in range(2):
                    nc.tensor.matmul(
                        w1ps[0:1, ch * 512:(ch + 1) * 512],
                        wa1[i][:],
                        hT[i][:, ch * 512:(ch + 1) * 512],
                        start=(i == 0),
                        stop=(i == 1),
                    )
            w1row = const.tile([1, N], DT, name="w1row", tag="w1row")
            nc.vector.tensor_copy(w1row[:], w1ps[:])
            wh1b = const.tile([128, N], DT, name="wh1b", tag="wh1b")
            nc.gpsimd.partition_broadcast(wh1b[:], w1row[0:1, :])
            wh1b02 = const.tile([128, N], DT, name="wh1b02", tag="wh1b02")
            nc.vector.tensor_scalar_mul(wh1b02[:], wh1b[:], ALPHA)

            den = const.tile([128, NT], DT, name="den", tag="den")
            outp = pbig.tile([FOUT, N], DT, name="big", tag="big")

            rep_cms = (
                [tc.For_i(0, loop_k, 1)] if loop_k
                else [contextlib.nullcontext() for _ in range(reps)]
            )
            for rep_cm in rep_cms:
                ctx_val = rep_cm.__enter__()
                for mt in range(NT):
                    ms = mt * 128
                    # mm1: Wh tile [128m, 66] = [Wh | Wh@a2 | 0.2*Wh@a2]
                    whps = psmall.tile([128, FOUT + 2], DT, name="whps", tag="whps")
                    for i in range(2):
                        nc.tensor.matmul(
                            whps[:, 0:FOUT + 2],
                            hT[i][:, ms:ms + 128],
                            wab[i][:],
                            start=(i == 0),
                            stop=(i == 1),
                        )
                    wh2 = wsm.tile([128, 2], DT, name="wh2", tag="wh2")
                    nc.vector.tensor_copy(wh2[:], whps[:, FOUT:FOUT + 2])

                    # leaky(e) with e = Wh1[n] + Wh2[m]
                    att = watt.tile([128, N], DT, name="att", tag="att")
                    nc.scalar.activation(
                        att[:, 0:C_ACT], wh1b[:, 0:C_ACT], AF.Prelu,
                        bias=wh2[:, 0:1], scale=1.0, alpha=ALPHA,
                    )
                    ut = wut.tile([128, N - C_ACT], DT, name="ut", tag="ut")
                    nc.vector.tensor_scalar_add(ut[:], wh1b02[:, C_ACT:], wh2[:, 1:2])
                    nc.vector.scalar_tensor_tensor(
                        att[:, C_ACT:], wh1b[:, C_ACT:], wh2[:, 0:1], ut[:],
                        op0=ALU.add, op1=ALU.max,
                    )

                    # mask: att += {0, -57344} (fp8 -> f32 cast + add in DMA)
                    nc.gpsimd.dma_start(
                        att[:], adj8_d.ap()[ms:ms + 128, :], accum_op=ALU.add,
                    )

                    # P = exp(att), den = row-sum
                    ptile = wp.tile([128, N], DT, name="pt", tag="pt")
                    nc.scalar.activation(
                        ptile[:], att[:], AF.Exp, accum_out=den[:, mt:mt + 1],
                    )

                    # fold 1/den into Wh
                    rc = wsm.tile([128, 1], DT, name="rc", tag="rc")
                    nc.vector.reciprocal(rc[:], den[:, mt:mt + 1])
                    whp = wsm.tile([128, FOUT], DT, name="whp", tag="whp")
                    nc.vector.tensor_scalar_mul(whp[:], whps[:, 0:FOUT], rc[:, 0:1])

                    # mm2: out_T[o, n] += whp.T @ P
                    for ch in range(4):
                        nc.tensor.matmul(
                            outp[:, ch * 512:(ch + 1) * 512],
                            whp[:],
                            ptile[:, ch * 512:(ch + 1) * 512],
                            start=(mt == 0),
                            stop=(mt == NT - 1),
                        )

                # ---- ELU tail: elu(x) = relu(x) + min(exp(x)-1, 0) ----
                t_ = wout.tile([FOUT, N], DT, name="t", tag="t")
                r_ = wout.tile([FOUT, N], DT, name="r", tag="r")
                q_ = wout.tile([FOUT, N], DT, name="q", tag="q")
                osb = wout.tile([FOUT, N], DT, name="osb", tag="osb")
                nc.scalar.activation(t_[:], outp[:], AF.Exp)
                nc.scalar.activation(r_[:], outp[:], AF.Relu)
                nc.vector.tensor_scalar(
                    q_[:], t_[:], -1.0, 0.0, op0=ALU.add, op1=ALU.min,
                )
                nc.vector.tensor_tensor(osb[:], r_[:], q_[:], op=ALU.add)
                nc.sync.dma_start(out_d.ap(), osb[:])
                rep_cm.__exit__(None, None, None)

    nc.compile()
    return nc


def prepare_in_maps(h, adj, W, a):
    in_maps = []
    for b in range(B):
        hT = np.ascontiguousarray(h[b].T)
        adj8 = ((adj[b].T.astype(np.float32) - 1.0) * MASK8).astype(
            ml_dtypes.float8_e5m2
        )
        arow = np.ascontiguousarray(a[b].reshape(1, 2 * FOUT).astype(np.float32))
        in_maps.append(
            {
                "hT": hT,
                "W": np.ascontiguousarray(W[b]),
                "arow": arow,
                "adj8": adj8,
            }
        )
    return in_maps


def kernel(h, adj, W, a):
    """Full-input entry point: returns elu-GAT output [8, 2048, 64] float32."""
    if "nc" not in _CACHE:
        _CACHE["nc"] = build_program()
    nc = _CACHE["nc"]
    in_maps = prepare_in_maps(h, adj, W, a)
    res = bass_utils.run_bass_kernel_spmd(nc, in_maps, core_ids=list(range(B)))
    out = np.stack([res.results[b]["out"].T for b in range(B)])
    return np.ascontiguousarray(out.astype(np.float32))

